# revision 3
# baseline (speedup 1.0000x reference)
"""Bahdanau (additive) attention fused Trainium2 kernel.

Reference computation (per batch n):
    qc      = tanh(query[q,e] + context[v,e])            # [NQ, NV, NE]
    logits  = einsum('qve,e->qv', qc, w_logit) + b_logit
    probs   = softmax(logits / temp, axis=v)
    heads   = leaky_relu(probs @ memory, 0.01)           # [NQ, NE]
    out     = heads @ w_reduce.T + b_reduce              # [NQ, NE]

Sharding: 8 cores = 4 batches x 2 query-halves (data parallel, no
collectives).  Each core handles n = core//2, q-range = (core%2)*128..+128.

Per-core algorithm (all fp32):
  - layout: e on partitions (2 tiles of 128), v on free dim.
  - DVE tensor_scalar_add broadcasts query bias onto context rows
    (2x perf mode), producing pre-activation S tiles [128, 512].
  - ACT applies tanh on grouped tiles (big free dims amortize the fixed
    per-op SBUF latency).  This engine is the roofline (~109us/core).
  - PE reduces over e with one-hot-expanded w_logit columns (M=32,
    tile_position col groups) accumulating logits [q=128, v=512] in PSUM.
  - softmax: ACT exp with accum_out row-sum; DVE reciprocal.
  - PE transposes exp -> [v, q]; matmul with memory -> heads; Lrelu;
    transpose; matmul with w_reduce.T; scale by 1/rowsum; add b_reduce.

Host-side folds: w' = w_logit/temp (softmax temperature), b_logit dropped
(softmax shift invariance), w_reduce pre-transposed, b_reduce broadcast.
"""

import sys

for _p in ("/opt/trn_rl_repo",):
    if _p not in sys.path:
        sys.path.insert(0, _p)

from contextlib import ExitStack

import numpy as np

import concourse.bass as bass
import concourse.tile as tile
from concourse import bacc, mybir
from concourse import bass_utils

F32 = mybir.dt.float32

N, NQ, NV, NE = 4, 256, 512, 256
NCORES = 8
QH = NQ // 2          # queries per core
ET = NE // 128        # e tiles (partition dim)
VB = NV // 128        # v blocks of 128
G = 4                 # queries per ACT tanh group


def build_kernel_body(tc, ins, outs):
    nc = tc.nc
    ctxT_d, qT_d, mem_d, wpad_d, wrT_d, brb_d, ident_d = (
        ins["ctxT"], ins["qT"], ins["mem"], ins["wpad"], ins["wrT"],
        ins["brb"], ins["ident"],
    )
    out_d = outs["out"]

    with ExitStack() as ctx:
        consts = ctx.enter_context(tc.tile_pool(name="consts", bufs=1))
        spool = ctx.enter_context(tc.tile_pool(name="spool", bufs=2))
        tpool = ctx.enter_context(tc.tile_pool(name="tpool", bufs=2))
        small = ctx.enter_context(tc.tile_pool(name="small", bufs=1))
        pslog = ctx.enter_context(tc.tile_pool(name="pslog", bufs=1, space="PSUM"))
        pstr = ctx.enter_context(tc.tile_pool(name="pstr", bufs=2, space="PSUM"))
        psmm = ctx.enter_context(tc.tile_pool(name="psmm", bufs=1, space="PSUM"))

        # ---- constant loads --------------------------------------------
        ctxT_sb = consts.tile([128, ET, NV], F32)
        nc.sync.dma_start(out=ctxT_sb, in_=ctxT_d.rearrange("t p v -> p t v"))
        qT_sb = consts.tile([128, ET, QH], F32)
        nc.sync.dma_start(out=qT_sb, in_=qT_d.rearrange("t p q -> p t q"))
        mem_sb = consts.tile([128, VB, NE], F32)
        nc.sync.dma_start(out=mem_sb, in_=mem_d.rearrange("t p e -> p t e"))
        wpad_sb = consts.tile([128, ET, 32, 32], F32)
        nc.sync.dma_start(out=wpad_sb, in_=wpad_d)
        wrT_sb = consts.tile([128, ET, NE], F32)
        nc.sync.dma_start(out=wrT_sb, in_=wrT_d.rearrange("t p o -> p t o"))
        brb_sb = consts.tile([128, NE], F32)
        nc.sync.dma_start(out=brb_sb, in_=brb_d)
        ident_sb = consts.tile([128, 128], F32)
        nc.sync.dma_start(out=ident_sb, in_=ident_d)

        # ---- main loop: tanh + logits ----------------------------------
        logits_ps = pslog.tile([128, NV], F32)

        for g in range(QH // G):
            sgrp = spool.tile([128, G, ET, NV], F32)
            for i in range(G):
                q = g * G + i
                for t in range(ET):
                    nc.vector.tensor_scalar_add(
                        sgrp[:, i, t, :],
                        ctxT_sb[:, t, :],
                        qT_sb[:, t, q:q + 1],
                    )
            tgrp = tpool.tile([128, G, ET, NV], F32)
            nc.scalar.activation(
                out=tgrp.rearrange("p a t v -> p (a t v)"),
                in_=sgrp.rearrange("p a t v -> p (a t v)"),
                func=mybir.ActivationFunctionType.Tanh,
            )
            for i in range(G):
                q = g * G + i
                j, s = divmod(q, 32)
                for t in range(ET):
                    nc.tensor.matmul(
                        out=logits_ps[32 * j:32 * (j + 1), :],
                        lhsT=wpad_sb[:, t, s, :],
                        rhs=tgrp[:, i, t, :],
                        start=(s == 0 and t == 0),
                        stop=(s == 31 and t == ET - 1),
                        tile_position=(0, 32 * j),
                    )

        # ---- softmax ----------------------------------------------------
        expv = small.tile([128, NV], F32)
        rowsum = small.tile([128, 1], F32)
        nc.scalar.activation(
            out=expv, in_=logits_ps,
            func=mybir.ActivationFunctionType.Exp,
            accum_out=rowsum,
        )
        recip = small.tile([128, 1], F32)
        nc.vector.reciprocal(recip, rowsum)

        # ---- transpose exp -> [v, q] ------------------------------------
        expT = small.tile([128, VB, 128], F32)
        for vb in range(VB):
            tps = pstr.tile([128, 128], F32, name=f"tps{vb}", tag="tps")
            nc.tensor.transpose(tps, expv[:, 128 * vb:128 * (vb + 1)], ident_sb)
            nc.vector.tensor_copy(expT[:, vb, :], tps)

        # ---- heads = exp.T @ memory ------------------------------------
        heads_ps = psmm.tile([128, NE], F32, tag="ps2")
        for vb in range(VB):
            nc.tensor.matmul(
                out=heads_ps,
                lhsT=expT[:, vb, :],
                rhs=mem_sb[:, vb, :],
                start=(vb == 0),
                stop=(vb == VB - 1),
            )

        # ---- leaky relu: max(x, 0.01x) (recip scaling folded to end) ----
        h001 = small.tile([128, NE], F32)
        nc.vector.tensor_scalar_mul(h001, heads_ps, 0.01)
        hb = small.tile([128, NE], F32)
        nc.vector.tensor_max(hb, h001, heads_ps)

        # ---- transpose heads -> [e, q] ----------------------------------
        hbT = small.tile([128, ET, 128], F32)
        for eb in range(ET):
            tps2 = pstr.tile([128, 128], F32, name=f"tps2_{eb}", tag="tps")
            nc.tensor.transpose(tps2, hb[:, 128 * eb:128 * (eb + 1)], ident_sb)
            nc.vector.tensor_copy(hbT[:, eb, :], tps2)

        # ---- out = heads @ w_reduce.T -----------------------------------
        out_ps = psmm.tile([128, NE], F32, tag="ps3")
        for eb in range(ET):
            nc.tensor.matmul(
                out=out_ps,
                lhsT=hbT[:, eb, :],
                rhs=wrT_sb[:, eb, :],
                start=(eb == 0),
                stop=(eb == ET - 1),
            )

        # ---- scale by 1/rowsum, add bias, store -------------------------
        outsb = small.tile([128, NE], F32)
        nc.vector.tensor_scalar_mul(outsb, out_ps, recip)
        nc.vector.tensor_add(outsb, outsb, brb_sb)
        nc.sync.dma_start(out=out_d, in_=outsb)


_CACHE = {}


def build_program():
    if "nc" in _CACHE:
        return _CACHE["nc"]
    nc = bacc.Bacc(
        "TRN2", target_bir_lowering=False, debug=False, num_devices=NCORES
    )
    ins = {
        "ctxT": nc.dram_tensor("ctxT", [ET, 128, NV], F32, kind="ExternalInput").ap(),
        "qT": nc.dram_tensor("qT", [ET, 128, QH], F32, kind="ExternalInput").ap(),
        "mem": nc.dram_tensor("mem", [VB, 128, NE], F32, kind="ExternalInput").ap(),
        "wpad": nc.dram_tensor("wpad", [128, ET, 32, 32], F32, kind="ExternalInput").ap(),
        "wrT": nc.dram_tensor("wrT", [ET, 128, NE], F32, kind="ExternalInput").ap(),
        "brb": nc.dram_tensor("brb", [128, NE], F32, kind="ExternalInput").ap(),
        "ident": nc.dram_tensor("ident", [128, 128], F32, kind="ExternalInput").ap(),
    }
    outs = {
        "out": nc.dram_tensor("out", [QH, NE], F32, kind="ExternalOutput").ap(),
    }
    with tile.TileContext(nc) as tc:
        build_kernel_body(tc, ins, outs)
    nc.compile()
    _CACHE["nc"] = nc
    return nc


def make_in_maps(query, context, memory, w_logit, b_logit, temp, w_reduce,
                 b_reduce):
    query = np.asarray(query, np.float32)
    context = np.asarray(context, np.float32)
    memory = np.asarray(memory, np.float32)
    w_logit = np.asarray(w_logit, np.float32)
    temp = np.asarray(temp, np.float32)
    w_reduce = np.asarray(w_reduce, np.float32)
    b_reduce = np.asarray(b_reduce, np.float32)

    w_scaled = (w_logit / temp).astype(np.float32)          # fold temperature
    # one-hot expanded w columns: wpad[p, t, s, c] = w_scaled[t*128+p]*(c==s)
    wpad = np.zeros((128, ET, 32, 32), np.float32)
    ar = np.arange(32)
    for t in range(ET):
        wpad[:, t, ar, ar] = w_scaled[t * 128:(t + 1) * 128][:, None]
    wrT = np.ascontiguousarray(w_reduce.T).reshape(ET, 128, NE)
    brb = np.ascontiguousarray(np.broadcast_to(b_reduce, (128, NE)))
    ident = np.eye(128, dtype=np.float32)

    in_maps = []
    for c in range(NCORES):
        n, h = divmod(c, 2)
        ctxT = np.ascontiguousarray(context[n].T).reshape(ET, 128, NV)
        qT = np.ascontiguousarray(
            query[n, h * QH:(h + 1) * QH].T).reshape(ET, 128, QH)
        mem = np.ascontiguousarray(memory[n]).reshape(VB, 128, NE)
        in_maps.append({
            "ctxT": ctxT, "qT": qT, "mem": mem, "wpad": wpad,
            "wrT": wrT, "brb": brb, "ident": ident,
        })
    return in_maps


def gather_output(results):
    out = np.empty((N, NQ, NE), np.float32)
    for c in range(NCORES):
        n, h = divmod(c, 2)
        out[n, h * QH:(h + 1) * QH] = results[c]["out"]
    return out


def kernel(query, context, memory, w_logit, b_logit, temp, w_reduce,
           b_reduce, _trace=False):
    nc = build_program()
    in_maps = make_in_maps(query, context, memory, w_logit, b_logit, temp,
                           w_reduce, b_reduce)
    res = bass_utils.run_bass_kernel_spmd(
        nc, in_maps, core_ids=list(range(NCORES)), trace=_trace,
    )
    out = gather_output(res.results)
    if _trace:
        return out, res
    return out


if __name__ == "__main__":
    rng = np.random.default_rng(0)
    inputs = {
        "query": rng.standard_normal((N, NQ, NE), np.float32),
        "context": rng.standard_normal((N, NV, NE), np.float32),
        "memory": rng.standard_normal((N, NV, NE), np.float32),
        "w_logit": rng.standard_normal(NE, np.float32) / 16.0,
        "b_logit": np.float32(0.0),
        "temp": np.float32(1.0),
        "w_reduce": rng.standard_normal((NE, NE), np.float32) / 16.0,
        "b_reduce": np.zeros(NE, np.float32),
    }
    out = kernel(**inputs)
    print("out", out.shape, out.dtype, float(np.abs(out).mean()))


# revision 7
# speedup vs baseline: 1.6654x; 1.6654x over previous
"""Bahdanau (additive) attention fused Trainium2 kernel.

Reference computation (per batch n):
    qc      = tanh(query[q,e] + context[v,e])            # [NQ, NV, NE]
    logits  = einsum('qve,e->qv', qc, w_logit) + b_logit
    probs   = softmax(logits / temp, axis=v)
    heads   = leaky_relu(probs @ memory, 0.01)           # [NQ, NE]
    out     = heads @ w_reduce.T + b_reduce              # [NQ, NE]

Sharding: 8 cores = 4 batches x 2 query-halves (data parallel, no
collectives).  Each core handles n = core//2, q-range = (core%2)*128..+128.

Per-core algorithm (all fp32):
  - layout: e on partitions (2 tiles of 128), v on free dim.
  - DVE tensor_scalar_add broadcasts query bias onto context rows
    (2x perf mode), producing pre-activation S tiles [128, 512].
  - ACT applies tanh on grouped tiles (big free dims amortize the fixed
    per-op SBUF latency).  This engine is the roofline (~109us/core).
  - PE reduces over e with one-hot-expanded w_logit columns (M=32,
    tile_position col groups) accumulating logits [q=128, v=512] in PSUM.
  - softmax: ACT exp with accum_out row-sum; DVE reciprocal.
  - PE transposes exp -> [v, q]; matmul with memory -> heads; Lrelu;
    transpose; matmul with w_reduce.T; scale by 1/rowsum; add b_reduce.

Host-side folds: w' = w_logit/temp (softmax temperature), b_logit dropped
(softmax shift invariance), w_reduce pre-transposed, b_reduce broadcast.
"""

import sys

for _p in ("/opt/trn_rl_repo",):
    if _p not in sys.path:
        sys.path.insert(0, _p)

from contextlib import ExitStack

import numpy as np

import concourse.bass as bass
import concourse.tile as tile
from concourse import bacc, mybir
from concourse import bass_utils

F32 = mybir.dt.float32
F32R = mybir.dt.float32r

N, NQ, NV, NE = 4, 256, 512, 256
NCORES = 8
QH = NQ // 2          # queries per core
ET = NE // 128        # e tiles (partition dim)
VB = NV // 128        # v blocks of 128
G = 4                 # queries per ACT tanh group


def build_kernel_body(tc, ins, outs):
    nc = tc.nc
    ctxT_d, qT_d, mem_d, wpad_d, wrT_d, brb_d, ident_d = (
        ins["ctxT"], ins["qT"], ins["mem"], ins["wpad"], ins["wrT"],
        ins["brb"], ins["ident"],
    )
    out_d = outs["out"]

    with ExitStack() as ctx:
        consts = ctx.enter_context(tc.tile_pool(name="consts", bufs=1))
        spool = ctx.enter_context(tc.tile_pool(name="spool", bufs=2))
        tpool = ctx.enter_context(tc.tile_pool(name="tpool", bufs=2))
        small = ctx.enter_context(tc.tile_pool(name="small", bufs=1))
        pslog = ctx.enter_context(tc.tile_pool(name="pslog", bufs=1, space="PSUM"))
        pstr = ctx.enter_context(tc.tile_pool(name="pstr", bufs=2, space="PSUM"))
        psmm = ctx.enter_context(tc.tile_pool(name="psmm", bufs=1, space="PSUM"))

        # ---- constant loads --------------------------------------------
        ctxT_sb = consts.tile([128, ET, NV], F32)
        nc.sync.dma_start(out=ctxT_sb, in_=ctxT_d.rearrange("t p v -> p t v"))
        qT_sb = consts.tile([128, ET, QH], F32)
        nc.sync.dma_start(out=qT_sb, in_=qT_d.rearrange("t p q -> p t q"))
        mem_sb = consts.tile([128, VB, NE], F32)
        nc.sync.dma_start(out=mem_sb, in_=mem_d.rearrange("t p e -> p t e"))
        wpad_st = consts.tile([128, ET, 32, 32], F32)
        nc.sync.dma_start(out=wpad_st, in_=wpad_d)
        # rounded copy: f32r matmul operands must come from a rounding producer
        wpad_sb = consts.tile([128, ET, 32, 32], F32R)
        nc.vector.tensor_copy(
            wpad_sb.rearrange("p t s c -> p (t s c)"),
            wpad_st.rearrange("p t s c -> p (t s c)"),
        )
        wrT_sb = consts.tile([128, ET, NE], F32)
        nc.sync.dma_start(out=wrT_sb, in_=wrT_d.rearrange("t p o -> p t o"))
        brb_sb = consts.tile([128, NE], F32)
        nc.sync.dma_start(out=brb_sb, in_=brb_d)
        ident_sb = consts.tile([128, 128], F32)
        nc.sync.dma_start(out=ident_sb, in_=ident_d)

        # ---- main loop: tanh + logits ----------------------------------
        # f32r matmuls only support col-group 0, so logits live in four
        # [32, 512] PSUM tiles (q = 32j + s -> tile j, row s).
        lgs = [pslog.tile([32, NV], F32, name=f"lg{j}", tag=f"lg{j}")
               for j in range(4)]

        for g in range(QH // G):
            sgrp = spool.tile([128, G, ET, NV], F32)
            for i in range(G):
                q = g * G + i
                for t in range(ET):
                    nc.vector.tensor_scalar_add(
                        sgrp[:, i, t, :],
                        ctxT_sb[:, t, :],
                        qT_sb[:, t, q:q + 1],
                    )
            tgrp = tpool.tile([128, G, ET, NV], F32R)
            nc.scalar.activation(
                out=tgrp.rearrange("p a t v -> p (a t v)"),
                in_=sgrp.rearrange("p a t v -> p (a t v)"),
                func=mybir.ActivationFunctionType.Tanh,
            )
            for i in range(G):
                q = g * G + i
                j, s = divmod(q, 32)
                for t in range(ET):
                    nc.tensor.matmul(
                        out=lgs[j],
                        lhsT=wpad_sb[:, t, s, :],
                        rhs=tgrp[:, i, t, :],
                        start=(s == 0 and t == 0),
                        stop=(s == 31 and t == ET - 1),
                    )

        # ---- softmax (per 32-q block; probs scaled in place) ------------
        expvs = []
        for j in range(4):
            ev = small.tile([32, NV], F32, name=f"ev{j}")
            rs = small.tile([32, 1], F32, name=f"rs{j}")
            nc.scalar.activation(
                out=ev, in_=lgs[j],
                func=mybir.ActivationFunctionType.Exp,
                accum_out=rs,
            )
            rc = small.tile([32, 1], F32, name=f"rc{j}")
            nc.vector.reciprocal(rc, rs)
            nc.vector.tensor_scalar_mul(ev, ev, rc)
            expvs.append(ev)

        # ---- transpose probs -> [v, q] ----------------------------------
        expT = small.tile([128, VB, 128], F32)
        for vb in range(VB):
            tps = pstr.tile([128, 128], F32, name=f"tps{vb}", tag="tps")
            for j in range(4):
                nc.tensor.transpose(
                    tps[:, 32 * j:32 * (j + 1)],
                    expvs[j][:, 128 * vb:128 * (vb + 1)],
                    ident_sb[0:32, 0:32],
                )
            nc.vector.tensor_copy(expT[:, vb, :], tps)

        # ---- heads = exp.T @ memory ------------------------------------
        heads_ps = psmm.tile([128, NE], F32, tag="ps2")
        for vb in range(VB):
            nc.tensor.matmul(
                out=heads_ps,
                lhsT=expT[:, vb, :],
                rhs=mem_sb[:, vb, :],
                start=(vb == 0),
                stop=(vb == VB - 1),
            )

        # ---- leaky relu: max(x, 0.01x) (recip scaling folded to end) ----
        h001 = small.tile([128, NE], F32)
        nc.vector.tensor_scalar_mul(h001, heads_ps, 0.01)
        hb = small.tile([128, NE], F32)
        nc.vector.tensor_max(hb, h001, heads_ps)

        # ---- transpose heads -> [e, q] ----------------------------------
        hbT = small.tile([128, ET, 128], F32)
        for eb in range(ET):
            tps2 = pstr.tile([128, 128], F32, name=f"tps2_{eb}", tag="tps")
            nc.tensor.transpose(tps2, hb[:, 128 * eb:128 * (eb + 1)], ident_sb)
            nc.vector.tensor_copy(hbT[:, eb, :], tps2)

        # ---- out = heads @ w_reduce.T -----------------------------------
        out_ps = psmm.tile([128, NE], F32, tag="ps3")
        for eb in range(ET):
            nc.tensor.matmul(
                out=out_ps,
                lhsT=hbT[:, eb, :],
                rhs=wrT_sb[:, eb, :],
                start=(eb == 0),
                stop=(eb == ET - 1),
            )

        # ---- add bias, store --------------------------------------------
        outsb = small.tile([128, NE], F32)
        nc.vector.tensor_add(outsb, out_ps, brb_sb)
        nc.sync.dma_start(out=out_d, in_=outsb)


_CACHE = {}


def build_program():
    if "nc" in _CACHE:
        return _CACHE["nc"]
    nc = bacc.Bacc(
        "TRN2", target_bir_lowering=False, debug=False, num_devices=NCORES
    )
    ins = {
        "ctxT": nc.dram_tensor("ctxT", [ET, 128, NV], F32, kind="ExternalInput").ap(),
        "qT": nc.dram_tensor("qT", [ET, 128, QH], F32, kind="ExternalInput").ap(),
        "mem": nc.dram_tensor("mem", [VB, 128, NE], F32, kind="ExternalInput").ap(),
        "wpad": nc.dram_tensor("wpad", [128, ET, 32, 32], F32, kind="ExternalInput").ap(),
        "wrT": nc.dram_tensor("wrT", [ET, 128, NE], F32, kind="ExternalInput").ap(),
        "brb": nc.dram_tensor("brb", [128, NE], F32, kind="ExternalInput").ap(),
        "ident": nc.dram_tensor("ident", [128, 128], F32, kind="ExternalInput").ap(),
    }
    outs = {
        "out": nc.dram_tensor("out", [QH, NE], F32, kind="ExternalOutput").ap(),
    }
    with tile.TileContext(nc) as tc:
        build_kernel_body(tc, ins, outs)
    nc.compile()
    _CACHE["nc"] = nc
    return nc


def make_in_maps(query, context, memory, w_logit, b_logit, temp, w_reduce,
                 b_reduce):
    query = np.asarray(query, np.float32)
    context = np.asarray(context, np.float32)
    memory = np.asarray(memory, np.float32)
    w_logit = np.asarray(w_logit, np.float32)
    temp = np.asarray(temp, np.float32)
    w_reduce = np.asarray(w_reduce, np.float32)
    b_reduce = np.asarray(b_reduce, np.float32)

    w_scaled = (w_logit / temp).astype(np.float32)          # fold temperature
    # one-hot expanded w columns: wpad[p, t, s, c] = w_scaled[t*128+p]*(c==s)
    wpad = np.zeros((128, ET, 32, 32), np.float32)
    ar = np.arange(32)
    for t in range(ET):
        wpad[:, t, ar, ar] = w_scaled[t * 128:(t + 1) * 128][:, None]
    wrT = np.ascontiguousarray(w_reduce.T).reshape(ET, 128, NE)
    brb = np.ascontiguousarray(np.broadcast_to(b_reduce, (128, NE)))
    ident = np.eye(128, dtype=np.float32)

    in_maps = []
    for c in range(NCORES):
        n, h = divmod(c, 2)
        ctxT = np.ascontiguousarray(context[n].T).reshape(ET, 128, NV)
        qT = np.ascontiguousarray(
            query[n, h * QH:(h + 1) * QH].T).reshape(ET, 128, QH)
        mem = np.ascontiguousarray(memory[n]).reshape(VB, 128, NE)
        in_maps.append({
            "ctxT": ctxT, "qT": qT, "mem": mem, "wpad": wpad,
            "wrT": wrT, "brb": brb, "ident": ident,
        })
    return in_maps


def gather_output(results):
    out = np.empty((N, NQ, NE), np.float32)
    for c in range(NCORES):
        n, h = divmod(c, 2)
        out[n, h * QH:(h + 1) * QH] = results[c]["out"]
    return out


def kernel(query, context, memory, w_logit, b_logit, temp, w_reduce,
           b_reduce, _trace=False):
    nc = build_program()
    in_maps = make_in_maps(query, context, memory, w_logit, b_logit, temp,
                           w_reduce, b_reduce)
    res = bass_utils.run_bass_kernel_spmd(
        nc, in_maps, core_ids=list(range(NCORES)), trace=_trace,
    )
    out = gather_output(res.results)
    if _trace:
        return out, res
    return out


if __name__ == "__main__":
    rng = np.random.default_rng(0)
    inputs = {
        "query": rng.standard_normal((N, NQ, NE), np.float32),
        "context": rng.standard_normal((N, NV, NE), np.float32),
        "memory": rng.standard_normal((N, NV, NE), np.float32),
        "w_logit": rng.standard_normal(NE, np.float32) / 16.0,
        "b_logit": np.float32(0.0),
        "temp": np.float32(1.0),
        "w_reduce": rng.standard_normal((NE, NE), np.float32) / 16.0,
        "b_reduce": np.zeros(NE, np.float32),
    }
    out = kernel(**inputs)
    print("out", out.shape, out.dtype, float(np.abs(out).mean()))
